# revision 34
# baseline (speedup 1.0000x reference)
"""FBGCN layer on 8 Trainium2 NeuronCores.

Math (reference):
    Lhp = d_inv @ lap @ d_inv
    Hh  = Lhp @ relu(x @ W_high)
    Hl  = relu(gcn_conv(x, edge_index, W_conv, b_conv))
    out = aL * Hl + aH * Hh

Kernel strategy:
  * Re-associate the high-pass chain: Hh = d_inv @ (lap @ (d_inv @ u)),
    u = relu(x @ W_high) — three thin [N,N]@[N,256] matmuls instead of two
    N^3 matmuls (10x fewer FLOPs).
  * GCN scatter-add becomes a dense matmul agg = A @ xw where
    A[dst,src] = sum of edge norms (built on host from the indices; this is
    index preprocessing, the feature-payload compute stays on device).
  * aL is folded into A and b_conv (aL*relu(y) == relu(aL*y) for aL>=0);
    aH is folded into lap. No runtime scalars reach the device.
  * 1D row-shard over 8 cores; AllGather the thin intermediates (v, w)
    between stages; u/xw computed replicated (cheaper than a collective).
  * Matmul operands are fp16 (full PE rate, half the HBM traffic;
    fp32 PSUM accumulation). Big matrices are host-pre-tiled into
    [m_block, 128, 64, 512] so every kxm/kxn DMA is contiguous.
"""

import sys
import types
from contextlib import ExitStack

import numpy as np

N = 8192
E = 262144
D = 256
N_CORES = 8
RPC = N // N_CORES  # rows per core
MB = 512            # m-block width of the pre-tiled big matrices

LAST_EXEC_NS = None
LAST_RESULTS = None

_PROGRAM_CACHE = {}


def _patch_tile_drain(tile, mybir):
    """Split the Tile exit-drain's sem waits across multiple Drain
    instructions: this walrus build rejects >1 sync wait on a Drain op
    ("Too many sync wait commands")."""
    if getattr(tile.TileContext, "_drain_patched", False):
        return
    from concourse.vector_clock import ScopedClock

    def _patched(self, tick_clock, wait_clock):
        drain_inst = self.nc.sync.drain()
        wait_clock.add_sem_waits(
            drain_inst.ins, ScopedClock({None: tick_clock.global_clock})
        )
        si = drain_inst.ins.sync_info
        if si is not None and si.on_wait and len(si.on_wait) > 1:
            waits = list(si.on_wait)
            si.on_wait = waits[:1]
            for w in waits[1:]:
                extra = self.nc.sync.drain()
                extra.ins.sync_info = mybir.SyncInfo(on_wait=[w], on_update=[])
        self.nc.all_engine_barrier()
        popped = self.nc._tile_sem_poison_stack.pop()
        assert popped is self._sem_poison
        self.nc.clear_and_free_semaphores(list(self.sems.allocated().values()))
        self.nc.all_engine_barrier()

    tile.TileContext._drain_and_barrier = _patched
    tile.TileContext._drain_patched = True


def _split_excess_waits(nc, mybir, max_waits=1):
    """This walrus build rejects instructions carrying more than one sync
    wait ("Too many sync wait commands"). Hoist excess waits onto
    EventSemaphore carriers inserted just before the instruction on the
    same engine — serial execution makes this equivalent."""
    counter = [0]
    for fn in nc.m.functions:
        for blk in fn.blocks:
            insts = list(blk.instructions)
            if not any(
                i.sync_info is not None
                and i.sync_info.on_wait
                and len(i.sync_info.on_wait) > max_waits
                for i in insts
            ):
                continue
            new = []
            for inst in insts:
                si = inst.sync_info
                if si is not None and si.on_wait and len(si.on_wait) > max_waits:
                    waits = list(si.on_wait)
                    for w in waits[:-max_waits]:
                        counter[0] += 1
                        carrier = mybir.InstEventSemaphore(
                            name=f"wait_split_{counter[0]}", ins=[], outs=[]
                        )
                        carrier.engine = inst.engine
                        carrier.sync_info = mybir.SyncInfo(
                            on_wait=[w], on_update=[]
                        )
                        new.append(carrier)
                    si.on_wait = waits[-max_waits:]
                new.append(inst)
            blk.instructions = new
    return counter[0]


def _install_ntff_hook():
    """Register the axon NTFF profiling hook (the image's antenv package
    lacks axon_hooks; provide it so trace=True works)."""
    if "antenv.axon_hooks" in sys.modules:
        return
    try:
        import antenv
    except ImportError:
        antenv = types.ModuleType("antenv")
        sys.modules["antenv"] = antenv
    mod = types.ModuleType("antenv.axon_hooks")
    _state = {"h": None}
    mod.set_axon_ntff_profile_hook = lambda h: _state.__setitem__("h", h)
    mod.get_axon_ntff_profile_hook = lambda: _state["h"]
    sys.modules["antenv.axon_hooks"] = mod
    antenv.axon_hooks = mod
    try:
        from trn_agent_boot.trn_boot import _ntff_profile_via_ctypes

        mod.set_axon_ntff_profile_hook(
            _ntff_profile_via_ctypes("/opt/axon/libaxon_pjrt.so")
        )
    except Exception:
        pass


def _build_program():
    import concourse.bass as bass
    import concourse.mybir as mybir
    import concourse.tile as tile
    from concourse.kernels.tile_matmul import (
        ShapeInfo,
        composable_matmul_tile_kernel,
        dma_from_dram_kxm,
        dma_from_dram_kxn,
        dma_to_dram_mxn,
    )

    _patch_tile_drain(tile, mybir)

    f32 = mybir.dt.float32
    f16 = mybir.dt.float16
    groups = [list(range(N_CORES))]
    NBLK = RPC // MB      # m-blocks per core
    PO = N // 128         # outer-k per thin tensor
    POL = RPC // 128      # outer-k per local shard
    KS = 8                # K_SUBTILES at K_TILE=1024

    nc = bass.Bass("TRN2", target_bir_lowering=False, num_devices=N_CORES)

    xT = nc.dram_tensor("xT", [D, N], f16, kind="ExternalInput")
    Wcat = nc.dram_tensor("Wcat", [D, 2 * D], f16, kind="ExternalInput")
    bp = nc.dram_tensor("bp", [D, 1], f32, kind="ExternalInput")
    # big matrices pre-tiled: [m_block, pi, po, mi]; k = po*128 + pi
    dT4 = nc.dram_tensor("dT4", [NBLK, 128, PO, MB], f16, kind="ExternalInput")
    lapT4 = nc.dram_tensor("lapT4", [NBLK, 128, PO, MB], f16, kind="ExternalInput")
    AT4 = nc.dram_tensor("AT4", [NBLK, 128, PO, MB], f16, kind="ExternalInput")
    outT = nc.dram_tensor("outT", [D, RPC], f32, kind="ExternalOutput")

    THIN_SHAPE = ShapeInfo(pdims=((128, PO),), fdims=(D,))

    with tile.TileContext(nc) as tc:
        with ExitStack() as stk:
            dram = stk.enter_context(tc.tile_pool(name="dram", bufs=1, space="DRAM"))
            v_c4 = dram.tile([128, POL, D], f16)
            w_c4 = dram.tile([128, POL, D], f16)
            v_sh = dram.tile([N_CORES, 128, POL, D], f16, addr_space="Shared")
            w_sh = dram.tile([N_CORES, 128, POL, D], f16, addr_space="Shared")
            aggT = dram.tile([D, RPC], f32)

            # thin operands live entirely in SBUF: [pi, po, f], row = po*128+pi
            u_sb, free_u = tc.tile([128, PO, D], f16, name="u_sb")
            xw_sb, free_xw = tc.tile([128, PO, D], f16, name="xw_sb")
            v_sb, free_v = tc.tile([128, PO, D], f16, name="v_sb")
            w_sb, free_w = tc.tile([128, PO, D], f16, name="w_sb")
            _frees = [free_w, free_v, free_xw, free_u]  # LIFO release order

            def allgather(src, dst, dst_sb):
                nc.gpsimd.collective_compute(
                    "AllGather",
                    mybir.AluOpType.bypass,
                    replica_groups=groups,
                    ins=[src.opt()],
                    outs=[dst.opt()],
                )
                # unpack [c, pi, po_l, f] -> SBUF [pi, c*POL + po_l, f]
                for c in range(N_CORES):
                    nc.sync.dma_start(dst_sb[:, c * POL : (c + 1) * POL, :], dst[c])

            def thin_kxn_producer(sb):
                def produce(nc_, md):
                    return sb[:, md.k_tile_idx * KS : (md.k_tile_idx + 1) * KS, :]

                return produce

            thin_kxm_producer = thin_kxn_producer  # same slicing, TileKxM md

            def sbuf_mxn_producer(sb):
                def produce(nc_, md):
                    t = md.m_tile_idx
                    return sb[:, t * (MB // 128) : (t + 1) * (MB // 128), :]

                return produce

            def relu_evict(nc_, psum, sbuf, md):
                nc_.scalar.activation(
                    sbuf[:], psum[:], mybir.ActivationFunctionType.Relu
                )

            def noop_consumer(nc_, sbuf, md):
                pass

            with (
                tc.tile_pool(name="xT_pool", bufs=3) as pool_xt,
                tc.tile_pool(name="w_pool", bufs=2) as pool_w,
            ):
                # u = relu(x @ W_high), xw = x @ W_conv — evicted straight
                # into SBUF (replicated on every core; no DRAM round trip).
                for sb, wcols, reducer in (
                    (u_sb, slice(0, D), relu_evict),
                    (xw_sb, slice(D, 2 * D), None),
                ):
                    kxm_p, kxm_shape = dma_from_dram_kxm(pool_xt, xT[:])
                    kxn_p, kxn_shape = dma_from_dram_kxn(pool_w, Wcat[:, wcols])
                    composable_matmul_tile_kernel(
                        tc=tc,
                        kxm_shape=kxm_shape,
                        kxn_shape=kxn_shape,
                        output_type=None,
                        kxm_producer=kxm_p,
                        kxn_producer=kxn_p,
                        mxn_consumer=noop_consumer,
                        mxn_subtile_producer=sbuf_mxn_producer(sb),
                        **({"mxn_subtile_reducer": reducer} if reducer else {}),
                        cache_tiles=False,
                    )

            with tc.tile_pool(name="big_stream", bufs=5) as pool_big:

                def big_x_thin(big4, thin_sb, mxn3d):
                    # mxn3d[t]: [pi, po-slice, f] block of the output
                    for t in range(NBLK):
                        kxm_p, kxm_shape = dma_from_dram_kxm(pool_big, big4[t])
                        composable_matmul_tile_kernel(
                            tc=tc,
                            kxm_shape=kxm_shape,
                            kxn_shape=THIN_SHAPE,
                            output_type=f16,
                            kxm_producer=kxm_p,
                            kxn_producer=thin_kxn_producer(thin_sb),
                            mxn_consumer=dma_to_dram_mxn(mxn3d(t)),
                            MAX_K_TILE_SIZE=1024,
                            cache_tiles=False,
                        )

                def thin_x_big(thin_sb, big4, mxn_consumer_fn, blocks=None):
                    # out [D, RPC] fp32, one call per 512-wide column block
                    for t in blocks if blocks is not None else range(NBLK):
                        kxn_p, kxn_shape = dma_from_dram_kxn(pool_big, big4[t])
                        composable_matmul_tile_kernel(
                            tc=tc,
                            kxm_shape=THIN_SHAPE,
                            kxn_shape=kxn_shape,
                            output_type=f32,
                            kxm_producer=thin_kxm_producer(thin_sb),
                            kxn_producer=kxn_p,
                            mxn_consumer=mxn_consumer_fn(t),
                            MAX_K_TILE_SIZE=1024,
                            cache_tiles=False,
                        )

                agg_consumer = lambda t: dma_to_dram_mxn(
                    aggT[:, t * MB : (t + 1) * MB]
                )

                # v_c = d_inv[rows_c] @ u
                big_x_thin(
                    dT4, u_sb,
                    lambda t: v_c4[:, t * (MB // 128) : (t + 1) * (MB // 128), :],
                )

                # aggT = (aL*A[rows_c] @ xw).T = xw.T @ AT — independent of
                # v/w; block 0 is traced here to fill the first AllGather
                # bubble, block 1 after D to fill the second.
                thin_x_big(xw_sb, AT4, agg_consumer, blocks=[0])
                allgather(v_c4, v_sh, v_sb)

                # w_c = (aH * lap)[rows_c] @ v
                big_x_thin(
                    lapT4, v_sb,
                    lambda t: w_c4[:, t * (MB // 128) : (t + 1) * (MB // 128), :],
                )
                thin_x_big(xw_sb, AT4, agg_consumer, blocks=[1])
                allgather(w_c4, w_sh, w_sb)

                # hhT = (d_inv[rows_c] @ w).T = w.T @ dT, fused with the
                # final combine: outT = relu(aggT + b') + hhT.
                with tc.tile_pool(name="combine", bufs=2) as gp:
                    bp_sb, free_bp = tc.tile([128, 2, 1], f32, name="bp_sb")
                    nc.sync.dma_start(
                        bp_sb[:], bp[:].rearrange("(po pi) one -> pi po one", pi=128)
                    )

                    def combine_consumer(t):
                        def consume(nc_, sbuf, md):
                            # sbuf [128, 2, 512] f32 = hhT cols [t*MB,(t+1)*MB)
                            at = gp.tile([128, 2, MB], f32, name="cmb_at")
                            nc_.sync.dma_start(
                                at[:],
                                aggT[:, t * MB : (t + 1) * MB].rearrange(
                                    "(po pi) f -> pi po f", pi=128
                                ),
                            )
                            for s_ in range(2):
                                nc_.scalar.activation(
                                    at[:, s_, :],
                                    at[:, s_, :],
                                    mybir.ActivationFunctionType.Relu,
                                    bias=bp_sb[:, s_, :],
                                )
                            nc_.vector.tensor_add(at[:], at[:], sbuf[:])
                            nc_.sync.dma_start(
                                outT[:, t * MB : (t + 1) * MB].rearrange(
                                    "(po pi) f -> pi po f", pi=128
                                ),
                                at[:],
                            )

                        return consume

                    thin_x_big(w_sb, dT4, combine_consumer)
                    free_bp()

            for f in _frees:
                f()

    _split_excess_waits(nc, mybir)
    return nc


def _get_program():
    if "nc" not in _PROGRAM_CACHE:
        _PROGRAM_CACHE["nc"] = _build_program()
    return _PROGRAM_CACHE["nc"]


def _tile_big(mat_t):
    """[N, RPC] (k-major) -> [NBLK, 128, N//128, MB] fp16 so that each
    [pi, po-slice, :] kxm/kxn tile DMA is contiguous per partition."""
    m16 = np.asarray(mat_t, dtype=np.float16)
    nblk = RPC // MB
    return np.ascontiguousarray(
        m16.reshape(N // 128, 128, nblk, MB).transpose(2, 1, 0, 3)
    )


def _host_prep(x, edge_index, lap, d_inv, W_high, W_conv, b_conv, aL, aH):
    x = np.asarray(x, dtype=np.float32)
    edge_index = np.asarray(edge_index)
    lap = np.asarray(lap, dtype=np.float32)
    d_inv = np.asarray(d_inv, dtype=np.float32)
    W_high = np.asarray(W_high, dtype=np.float32)
    W_conv = np.asarray(W_conv, dtype=np.float32)
    b_conv = np.asarray(b_conv, dtype=np.float32)
    aL = float(np.asarray(aL).reshape(-1)[0])
    aH = float(np.asarray(aH).reshape(-1)[0])

    n = x.shape[0]
    src = edge_index[0].astype(np.int64)
    dst = edge_index[1].astype(np.int64)
    loops = np.arange(n, dtype=np.int64)
    src_all = np.concatenate([src, loops])
    dst_all = np.concatenate([dst, loops])

    deg = np.bincount(dst_all, minlength=n).astype(np.float32)
    dis = np.where(deg > 0, 1.0 / np.sqrt(np.maximum(deg, 1.0)), 0.0).astype(
        np.float32
    )
    # aL folded into the adjacency (aL*relu(y) == relu(aL*y), aL >= 0)
    norm_all = (dis[src_all] * dis[dst_all] * np.float32(aL)).astype(np.float32)

    AT_full = np.zeros((n, n), dtype=np.float32)  # AT[src, dst]
    np.add.at(AT_full, (src_all, dst_all), norm_all)

    bprime = (np.float32(aL) * b_conv).reshape(D, 1).astype(np.float32)
    xT16 = np.ascontiguousarray(x.T.astype(np.float16))  # [256, N]
    Wcat16 = np.ascontiguousarray(
        np.concatenate([W_high, W_conv], axis=1).astype(np.float16)
    )

    in_maps = []
    for c in range(N_CORES):
        rows = slice(c * RPC, (c + 1) * RPC)
        in_maps.append(
            {
                "xT": xT16,
                "Wcat": Wcat16,
                "bp": bprime,
                "dT4": _tile_big(d_inv[rows, :].T),
                "lapT4": _tile_big(lap[rows, :].T * np.float32(aH)),
                "AT4": _tile_big(AT_full[:, rows]),
            }
        )
    return in_maps


def kernel(
    x,
    edge_index,
    lap,
    d_inv,
    W_high,
    W_conv,
    b_conv,
    aL,
    aH,
    _profile=False,
):
    global LAST_EXEC_NS, LAST_RESULTS
    from concourse.bass_utils import run_bass_kernel_spmd

    if _profile:
        _install_ntff_hook()

    in_maps = _host_prep(
        x, edge_index, lap, d_inv, W_high, W_conv, b_conv, aL, aH
    )
    nc = _get_program()
    res = run_bass_kernel_spmd(
        nc, in_maps, list(range(N_CORES)), trace=bool(_profile)
    )
    LAST_EXEC_NS = res.exec_time_ns
    LAST_RESULTS = res
    out = np.concatenate(
        [res.results[c]["outT"].T for c in range(N_CORES)], axis=0
    )
    return np.ascontiguousarray(out.astype(np.float32))


# revision 35
# speedup vs baseline: 1.0192x; 1.0192x over previous
"""FBGCN layer on 8 Trainium2 NeuronCores.

Math (reference):
    Lhp = d_inv @ lap @ d_inv
    Hh  = Lhp @ relu(x @ W_high)
    Hl  = relu(gcn_conv(x, edge_index, W_conv, b_conv))
    out = aL * Hl + aH * Hh

Kernel strategy:
  * Re-associate the high-pass chain: Hh = d_inv @ (lap @ (d_inv @ u)),
    u = relu(x @ W_high) — three thin [N,N]@[N,256] matmuls instead of two
    N^3 matmuls (10x fewer FLOPs).
  * GCN scatter-add becomes a dense matmul agg = A @ xw where
    A[dst,src] = sum of edge norms (built on host from the indices; this is
    index preprocessing, the feature-payload compute stays on device).
  * aL is folded into A and b_conv (aL*relu(y) == relu(aL*y) for aL>=0);
    aH is folded into lap. No runtime scalars reach the device.
  * 1D row-shard over 8 cores; AllGather the thin intermediates (v, w)
    between stages; u/xw computed replicated (cheaper than a collective).
  * Matmul operands are fp16 (full PE rate, half the HBM traffic;
    fp32 PSUM accumulation). Big matrices are host-pre-tiled into
    [m_block, 128, 64, 512] so every kxm/kxn DMA is contiguous.
"""

import sys
import types
from contextlib import ExitStack

import numpy as np

N = 8192
E = 262144
D = 256
N_CORES = 8
RPC = N // N_CORES  # rows per core
MB = 512            # m-block width of the pre-tiled big matrices

LAST_EXEC_NS = None
LAST_RESULTS = None

_PROGRAM_CACHE = {}


def _patch_tile_drain(tile, mybir):
    """Split the Tile exit-drain's sem waits across multiple Drain
    instructions: this walrus build rejects >1 sync wait on a Drain op
    ("Too many sync wait commands")."""
    if getattr(tile.TileContext, "_drain_patched", False):
        return
    from concourse.vector_clock import ScopedClock

    def _patched(self, tick_clock, wait_clock):
        drain_inst = self.nc.sync.drain()
        wait_clock.add_sem_waits(
            drain_inst.ins, ScopedClock({None: tick_clock.global_clock})
        )
        si = drain_inst.ins.sync_info
        if si is not None and si.on_wait and len(si.on_wait) > 1:
            waits = list(si.on_wait)
            si.on_wait = waits[:1]
            for w in waits[1:]:
                extra = self.nc.sync.drain()
                extra.ins.sync_info = mybir.SyncInfo(on_wait=[w], on_update=[])
        self.nc.all_engine_barrier()
        popped = self.nc._tile_sem_poison_stack.pop()
        assert popped is self._sem_poison
        self.nc.clear_and_free_semaphores(list(self.sems.allocated().values()))
        self.nc.all_engine_barrier()

    tile.TileContext._drain_and_barrier = _patched
    tile.TileContext._drain_patched = True


def _split_excess_waits(nc, mybir, max_waits=1):
    """This walrus build rejects instructions carrying more than one sync
    wait ("Too many sync wait commands"). Hoist excess waits onto
    EventSemaphore carriers inserted just before the instruction on the
    same engine — serial execution makes this equivalent."""
    counter = [0]
    for fn in nc.m.functions:
        for blk in fn.blocks:
            insts = list(blk.instructions)
            if not any(
                i.sync_info is not None
                and i.sync_info.on_wait
                and len(i.sync_info.on_wait) > max_waits
                for i in insts
            ):
                continue
            new = []
            for inst in insts:
                si = inst.sync_info
                if si is not None and si.on_wait and len(si.on_wait) > max_waits:
                    waits = list(si.on_wait)
                    for w in waits[:-max_waits]:
                        counter[0] += 1
                        carrier = mybir.InstEventSemaphore(
                            name=f"wait_split_{counter[0]}", ins=[], outs=[]
                        )
                        carrier.engine = inst.engine
                        carrier.sync_info = mybir.SyncInfo(
                            on_wait=[w], on_update=[]
                        )
                        new.append(carrier)
                    si.on_wait = waits[-max_waits:]
                new.append(inst)
            blk.instructions = new
    return counter[0]


def _install_ntff_hook():
    """Register the axon NTFF profiling hook (the image's antenv package
    lacks axon_hooks; provide it so trace=True works)."""
    if "antenv.axon_hooks" in sys.modules:
        return
    try:
        import antenv
    except ImportError:
        antenv = types.ModuleType("antenv")
        sys.modules["antenv"] = antenv
    mod = types.ModuleType("antenv.axon_hooks")
    _state = {"h": None}
    mod.set_axon_ntff_profile_hook = lambda h: _state.__setitem__("h", h)
    mod.get_axon_ntff_profile_hook = lambda: _state["h"]
    sys.modules["antenv.axon_hooks"] = mod
    antenv.axon_hooks = mod
    try:
        from trn_agent_boot.trn_boot import _ntff_profile_via_ctypes

        mod.set_axon_ntff_profile_hook(
            _ntff_profile_via_ctypes("/opt/axon/libaxon_pjrt.so")
        )
    except Exception:
        pass


def _build_program():
    import concourse.bass as bass
    import concourse.mybir as mybir
    import concourse.tile as tile
    from concourse.kernels.tile_matmul import (
        ShapeInfo,
        composable_matmul_tile_kernel,
        dma_from_dram_kxm,
        dma_from_dram_kxn,
        dma_to_dram_mxn,
    )

    _patch_tile_drain(tile, mybir)

    f32 = mybir.dt.float32
    f16 = mybir.dt.float16
    groups = [list(range(N_CORES))]
    NBLK = RPC // MB      # m-blocks per core
    PO = N // 128         # outer-k per thin tensor
    POL = RPC // 128      # outer-k per local shard
    KS = 8                # K_SUBTILES at K_TILE=1024

    nc = bass.Bass("TRN2", target_bir_lowering=False, num_devices=N_CORES)

    xT = nc.dram_tensor("xT", [D, N], f16, kind="ExternalInput")
    Wcat = nc.dram_tensor("Wcat", [D, 2 * D], f16, kind="ExternalInput")
    bp = nc.dram_tensor("bp", [D, 1], f32, kind="ExternalInput")
    # big matrices pre-tiled: [m_block, pi, po, mi]; k = po*128 + pi
    dT4 = nc.dram_tensor("dT4", [NBLK, 128, PO, MB], f16, kind="ExternalInput")
    lapT4 = nc.dram_tensor("lapT4", [NBLK, 128, PO, MB], f16, kind="ExternalInput")
    AT4 = nc.dram_tensor("AT4", [NBLK, 128, PO, MB], f16, kind="ExternalInput")
    outT = nc.dram_tensor("outT", [D, RPC], f32, kind="ExternalOutput")

    THIN_SHAPE = ShapeInfo(pdims=((128, PO),), fdims=(D,))

    with tile.TileContext(nc) as tc:
        with ExitStack() as stk:
            dram = stk.enter_context(tc.tile_pool(name="dram", bufs=1, space="DRAM"))
            v_c4 = dram.tile([128, POL, D], f16)
            w_c4 = dram.tile([128, POL, D], f16)
            v_sh = dram.tile([N_CORES, 128, POL, D], f16, addr_space="Shared")
            w_sh = dram.tile([N_CORES, 128, POL, D], f16, addr_space="Shared")
            aggT = dram.tile([D, RPC], f32)

            # thin operands live entirely in SBUF: [pi, po, f], row = po*128+pi
            u_sb, free_u = tc.tile([128, PO, D], f16, name="u_sb")
            xw_sb, free_xw = tc.tile([128, PO, D], f16, name="xw_sb")
            v_sb, free_v = tc.tile([128, PO, D], f16, name="v_sb")
            w_sb, free_w = tc.tile([128, PO, D], f16, name="w_sb")
            _frees = [free_w, free_v, free_xw, free_u]  # LIFO release order

            def allgather(src, dst, dst_sb):
                nc.gpsimd.collective_compute(
                    "AllGather",
                    mybir.AluOpType.bypass,
                    replica_groups=groups,
                    ins=[src.opt()],
                    outs=[dst.opt()],
                )
                # unpack [c, pi, po_l, f] -> SBUF [pi, c*POL + po_l, f]
                for c in range(N_CORES):
                    nc.sync.dma_start(dst_sb[:, c * POL : (c + 1) * POL, :], dst[c])

            def thin_kxn_producer(sb):
                def produce(nc_, md):
                    return sb[:, md.k_tile_idx * KS : (md.k_tile_idx + 1) * KS, :]

                return produce

            thin_kxm_producer = thin_kxn_producer  # same slicing, TileKxM md

            def sbuf_mxn_producer(sb):
                def produce(nc_, md):
                    t = md.m_tile_idx
                    return sb[:, t * (MB // 128) : (t + 1) * (MB // 128), :]

                return produce

            def relu_evict(nc_, psum, sbuf, md):
                nc_.scalar.activation(
                    sbuf[:], psum[:], mybir.ActivationFunctionType.Relu
                )

            def noop_consumer(nc_, sbuf, md):
                pass

            with (
                tc.tile_pool(name="xT_pool", bufs=3) as pool_xt,
                tc.tile_pool(name="w_pool", bufs=2) as pool_w,
            ):
                # u = relu(x @ W_high), xw = x @ W_conv — evicted straight
                # into SBUF (replicated on every core; no DRAM round trip).
                for sb, wcols, reducer in (
                    (u_sb, slice(0, D), relu_evict),
                    (xw_sb, slice(D, 2 * D), None),
                ):
                    kxm_p, kxm_shape = dma_from_dram_kxm(pool_xt, xT[:])
                    kxn_p, kxn_shape = dma_from_dram_kxn(pool_w, Wcat[:, wcols])
                    composable_matmul_tile_kernel(
                        tc=tc,
                        kxm_shape=kxm_shape,
                        kxn_shape=kxn_shape,
                        output_type=None,
                        kxm_producer=kxm_p,
                        kxn_producer=kxn_p,
                        mxn_consumer=noop_consumer,
                        mxn_subtile_producer=sbuf_mxn_producer(sb),
                        **({"mxn_subtile_reducer": reducer} if reducer else {}),
                        cache_tiles=False,
                    )

            with tc.tile_pool(name="big_stream", bufs=6) as pool_big:

                def big_x_thin(big4, thin_sb, mxn3d):
                    # mxn3d[t]: [pi, po-slice, f] block of the output
                    for t in range(NBLK):
                        kxm_p, kxm_shape = dma_from_dram_kxm(pool_big, big4[t])
                        composable_matmul_tile_kernel(
                            tc=tc,
                            kxm_shape=kxm_shape,
                            kxn_shape=THIN_SHAPE,
                            output_type=f16,
                            kxm_producer=kxm_p,
                            kxn_producer=thin_kxn_producer(thin_sb),
                            mxn_consumer=dma_to_dram_mxn(mxn3d(t)),
                            MAX_K_TILE_SIZE=1024,
                            cache_tiles=False,
                        )

                def thin_x_big(thin_sb, big4, mxn_consumer_fn, blocks=None):
                    # out [D, RPC] fp32, one call per 512-wide column block
                    for t in blocks if blocks is not None else range(NBLK):
                        kxn_p, kxn_shape = dma_from_dram_kxn(pool_big, big4[t])
                        composable_matmul_tile_kernel(
                            tc=tc,
                            kxm_shape=THIN_SHAPE,
                            kxn_shape=kxn_shape,
                            output_type=f32,
                            kxm_producer=thin_kxm_producer(thin_sb),
                            kxn_producer=kxn_p,
                            mxn_consumer=mxn_consumer_fn(t),
                            MAX_K_TILE_SIZE=1024,
                            cache_tiles=False,
                        )

                agg_consumer = lambda t: dma_to_dram_mxn(
                    aggT[:, t * MB : (t + 1) * MB]
                )

                # v_c = d_inv[rows_c] @ u
                big_x_thin(
                    dT4, u_sb,
                    lambda t: v_c4[:, t * (MB // 128) : (t + 1) * (MB // 128), :],
                )

                # aggT = (aL*A[rows_c] @ xw).T = xw.T @ AT — independent of
                # v/w; block 0 is traced here to fill the first AllGather
                # bubble, block 1 after D to fill the second.
                thin_x_big(xw_sb, AT4, agg_consumer, blocks=[0])
                allgather(v_c4, v_sh, v_sb)

                # w_c = (aH * lap)[rows_c] @ v
                big_x_thin(
                    lapT4, v_sb,
                    lambda t: w_c4[:, t * (MB // 128) : (t + 1) * (MB // 128), :],
                )
                thin_x_big(xw_sb, AT4, agg_consumer, blocks=[1])
                allgather(w_c4, w_sh, w_sb)

                # hhT = (d_inv[rows_c] @ w).T = w.T @ dT, fused with the
                # final combine: outT = relu(aggT + b') + hhT.
                with tc.tile_pool(name="combine", bufs=2) as gp:
                    bp_sb, free_bp = tc.tile([128, 2, 1], f32, name="bp_sb")
                    nc.sync.dma_start(
                        bp_sb[:], bp[:].rearrange("(po pi) one -> pi po one", pi=128)
                    )

                    def combine_consumer(t):
                        def consume(nc_, sbuf, md):
                            # sbuf [128, 2, 512] f32 = hhT cols [t*MB,(t+1)*MB)
                            at = gp.tile([128, 2, MB], f32, name="cmb_at")
                            nc_.sync.dma_start(
                                at[:],
                                aggT[:, t * MB : (t + 1) * MB].rearrange(
                                    "(po pi) f -> pi po f", pi=128
                                ),
                            )
                            for s_ in range(2):
                                nc_.scalar.activation(
                                    at[:, s_, :],
                                    at[:, s_, :],
                                    mybir.ActivationFunctionType.Relu,
                                    bias=bp_sb[:, s_, :],
                                )
                            nc_.vector.tensor_add(at[:], at[:], sbuf[:])
                            nc_.sync.dma_start(
                                outT[:, t * MB : (t + 1) * MB].rearrange(
                                    "(po pi) f -> pi po f", pi=128
                                ),
                                at[:],
                            )

                        return consume

                    thin_x_big(w_sb, dT4, combine_consumer)
                    free_bp()

            for f in _frees:
                f()

    _split_excess_waits(nc, mybir)
    return nc


def _get_program():
    if "nc" not in _PROGRAM_CACHE:
        _PROGRAM_CACHE["nc"] = _build_program()
    return _PROGRAM_CACHE["nc"]


def _tile_big(mat_t):
    """[N, RPC] (k-major) -> [NBLK, 128, N//128, MB] fp16 so that each
    [pi, po-slice, :] kxm/kxn tile DMA is contiguous per partition."""
    m16 = np.asarray(mat_t, dtype=np.float16)
    nblk = RPC // MB
    return np.ascontiguousarray(
        m16.reshape(N // 128, 128, nblk, MB).transpose(2, 1, 0, 3)
    )


def _host_prep(x, edge_index, lap, d_inv, W_high, W_conv, b_conv, aL, aH):
    x = np.asarray(x, dtype=np.float32)
    edge_index = np.asarray(edge_index)
    lap = np.asarray(lap, dtype=np.float32)
    d_inv = np.asarray(d_inv, dtype=np.float32)
    W_high = np.asarray(W_high, dtype=np.float32)
    W_conv = np.asarray(W_conv, dtype=np.float32)
    b_conv = np.asarray(b_conv, dtype=np.float32)
    aL = float(np.asarray(aL).reshape(-1)[0])
    aH = float(np.asarray(aH).reshape(-1)[0])

    n = x.shape[0]
    src = edge_index[0].astype(np.int64)
    dst = edge_index[1].astype(np.int64)
    loops = np.arange(n, dtype=np.int64)
    src_all = np.concatenate([src, loops])
    dst_all = np.concatenate([dst, loops])

    deg = np.bincount(dst_all, minlength=n).astype(np.float32)
    dis = np.where(deg > 0, 1.0 / np.sqrt(np.maximum(deg, 1.0)), 0.0).astype(
        np.float32
    )
    # aL folded into the adjacency (aL*relu(y) == relu(aL*y), aL >= 0)
    norm_all = (dis[src_all] * dis[dst_all] * np.float32(aL)).astype(np.float32)

    AT_full = np.zeros((n, n), dtype=np.float32)  # AT[src, dst]
    np.add.at(AT_full, (src_all, dst_all), norm_all)

    bprime = (np.float32(aL) * b_conv).reshape(D, 1).astype(np.float32)
    xT16 = np.ascontiguousarray(x.T.astype(np.float16))  # [256, N]
    Wcat16 = np.ascontiguousarray(
        np.concatenate([W_high, W_conv], axis=1).astype(np.float16)
    )

    in_maps = []
    for c in range(N_CORES):
        rows = slice(c * RPC, (c + 1) * RPC)
        in_maps.append(
            {
                "xT": xT16,
                "Wcat": Wcat16,
                "bp": bprime,
                "dT4": _tile_big(d_inv[rows, :].T),
                "lapT4": _tile_big(lap[rows, :].T * np.float32(aH)),
                "AT4": _tile_big(AT_full[:, rows]),
            }
        )
    return in_maps


def kernel(
    x,
    edge_index,
    lap,
    d_inv,
    W_high,
    W_conv,
    b_conv,
    aL,
    aH,
    _profile=False,
):
    global LAST_EXEC_NS, LAST_RESULTS
    from concourse.bass_utils import run_bass_kernel_spmd

    if _profile:
        _install_ntff_hook()

    in_maps = _host_prep(
        x, edge_index, lap, d_inv, W_high, W_conv, b_conv, aL, aH
    )
    nc = _get_program()
    res = run_bass_kernel_spmd(
        nc, in_maps, list(range(N_CORES)), trace=bool(_profile)
    )
    LAST_EXEC_NS = res.exec_time_ns
    LAST_RESULTS = res
    out = np.concatenate(
        [res.results[c]["outT"].T for c in range(N_CORES)], axis=0
    )
    return np.ascontiguousarray(out.astype(np.float32))


# revision 36
# speedup vs baseline: 1.0229x; 1.0036x over previous
"""FBGCN layer on 8 Trainium2 NeuronCores.

Math (reference):
    Lhp = d_inv @ lap @ d_inv
    Hh  = Lhp @ relu(x @ W_high)
    Hl  = relu(gcn_conv(x, edge_index, W_conv, b_conv))
    out = aL * Hl + aH * Hh

Kernel strategy:
  * Re-associate the high-pass chain: Hh = d_inv @ (lap @ (d_inv @ u)),
    u = relu(x @ W_high) — three thin [N,N]@[N,256] matmuls instead of two
    N^3 matmuls (10x fewer FLOPs).
  * GCN scatter-add becomes a dense matmul agg = A @ xw where
    A[dst,src] = sum of edge norms (built on host from the indices; this is
    index preprocessing, the feature-payload compute stays on device).
  * aL is folded into A and b_conv (aL*relu(y) == relu(aL*y) for aL>=0);
    aH is folded into lap. No runtime scalars reach the device.
  * 1D row-shard over 8 cores; AllGather the thin intermediates (v, w)
    between stages; u/xw computed replicated (cheaper than a collective).
  * Matmul operands are fp16 (full PE rate, half the HBM traffic;
    fp32 PSUM accumulation). Big matrices are host-pre-tiled into
    [m_block, 128, 64, 512] so every kxm/kxn DMA is contiguous.
"""

import sys
import types
from contextlib import ExitStack

import numpy as np

N = 8192
E = 262144
D = 256
N_CORES = 8
RPC = N // N_CORES  # rows per core
MB = 512            # m-block width of the pre-tiled big matrices

LAST_EXEC_NS = None
LAST_RESULTS = None

_PROGRAM_CACHE = {}


def _patch_tile_drain(tile, mybir):
    """Split the Tile exit-drain's sem waits across multiple Drain
    instructions: this walrus build rejects >1 sync wait on a Drain op
    ("Too many sync wait commands")."""
    if getattr(tile.TileContext, "_drain_patched", False):
        return
    from concourse.vector_clock import ScopedClock

    def _patched(self, tick_clock, wait_clock):
        drain_inst = self.nc.sync.drain()
        wait_clock.add_sem_waits(
            drain_inst.ins, ScopedClock({None: tick_clock.global_clock})
        )
        si = drain_inst.ins.sync_info
        if si is not None and si.on_wait and len(si.on_wait) > 1:
            waits = list(si.on_wait)
            si.on_wait = waits[:1]
            for w in waits[1:]:
                extra = self.nc.sync.drain()
                extra.ins.sync_info = mybir.SyncInfo(on_wait=[w], on_update=[])
        self.nc.all_engine_barrier()
        popped = self.nc._tile_sem_poison_stack.pop()
        assert popped is self._sem_poison
        self.nc.clear_and_free_semaphores(list(self.sems.allocated().values()))
        self.nc.all_engine_barrier()

    tile.TileContext._drain_and_barrier = _patched
    tile.TileContext._drain_patched = True


def _split_excess_waits(nc, mybir, max_waits=1):
    """This walrus build rejects instructions carrying more than one sync
    wait ("Too many sync wait commands"). Hoist excess waits onto
    EventSemaphore carriers inserted just before the instruction on the
    same engine — serial execution makes this equivalent."""
    counter = [0]
    for fn in nc.m.functions:
        for blk in fn.blocks:
            insts = list(blk.instructions)
            if not any(
                i.sync_info is not None
                and i.sync_info.on_wait
                and len(i.sync_info.on_wait) > max_waits
                for i in insts
            ):
                continue
            new = []
            for inst in insts:
                si = inst.sync_info
                if si is not None and si.on_wait and len(si.on_wait) > max_waits:
                    waits = list(si.on_wait)
                    for w in waits[:-max_waits]:
                        counter[0] += 1
                        carrier = mybir.InstEventSemaphore(
                            name=f"wait_split_{counter[0]}", ins=[], outs=[]
                        )
                        carrier.engine = inst.engine
                        carrier.sync_info = mybir.SyncInfo(
                            on_wait=[w], on_update=[]
                        )
                        new.append(carrier)
                    si.on_wait = waits[-max_waits:]
                new.append(inst)
            blk.instructions = new
    return counter[0]


def _install_ntff_hook():
    """Register the axon NTFF profiling hook (the image's antenv package
    lacks axon_hooks; provide it so trace=True works)."""
    if "antenv.axon_hooks" in sys.modules:
        return
    try:
        import antenv
    except ImportError:
        antenv = types.ModuleType("antenv")
        sys.modules["antenv"] = antenv
    mod = types.ModuleType("antenv.axon_hooks")
    _state = {"h": None}
    mod.set_axon_ntff_profile_hook = lambda h: _state.__setitem__("h", h)
    mod.get_axon_ntff_profile_hook = lambda: _state["h"]
    sys.modules["antenv.axon_hooks"] = mod
    antenv.axon_hooks = mod
    try:
        from trn_agent_boot.trn_boot import _ntff_profile_via_ctypes

        mod.set_axon_ntff_profile_hook(
            _ntff_profile_via_ctypes("/opt/axon/libaxon_pjrt.so")
        )
    except Exception:
        pass


def _build_program():
    import concourse.bass as bass
    import concourse.mybir as mybir
    import concourse.tile as tile
    from concourse.kernels.tile_matmul import (
        ShapeInfo,
        composable_matmul_tile_kernel,
        dma_from_dram_kxm,
        dma_from_dram_kxn,
        dma_to_dram_mxn,
    )

    _patch_tile_drain(tile, mybir)

    f32 = mybir.dt.float32
    f16 = mybir.dt.float16
    groups = [list(range(N_CORES))]
    NBLK = RPC // MB      # m-blocks per core
    PO = N // 128         # outer-k per thin tensor
    POL = RPC // 128      # outer-k per local shard
    KS = 8                # K_SUBTILES at K_TILE=1024

    nc = bass.Bass("TRN2", target_bir_lowering=False, num_devices=N_CORES)

    xT = nc.dram_tensor("xT", [D, N], f16, kind="ExternalInput")
    Wcat = nc.dram_tensor("Wcat", [D, 2 * D], f16, kind="ExternalInput")
    bp = nc.dram_tensor("bp", [D, 1], f32, kind="ExternalInput")
    # big matrices pre-tiled: [m_block, pi, po, mi]; k = po*128 + pi
    dT4 = nc.dram_tensor("dT4", [NBLK, 128, PO, MB], f16, kind="ExternalInput")
    lapT4 = nc.dram_tensor("lapT4", [NBLK, 128, PO, MB], f16, kind="ExternalInput")
    AT4 = nc.dram_tensor("AT4", [NBLK, 128, PO, MB], f16, kind="ExternalInput")
    outT = nc.dram_tensor("outT", [D, RPC], f32, kind="ExternalOutput")

    THIN_SHAPE = ShapeInfo(pdims=((128, PO),), fdims=(D,))

    with tile.TileContext(nc) as tc:
        with ExitStack() as stk:
            dram = stk.enter_context(tc.tile_pool(name="dram", bufs=1, space="DRAM"))
            v_c4 = dram.tile([128, POL, D], f16)
            w_c4 = dram.tile([128, POL, D], f16)
            v_sh = dram.tile([N_CORES, 128, POL, D], f16, addr_space="Shared")
            w_sh = dram.tile([N_CORES, 128, POL, D], f16, addr_space="Shared")
            aggT = dram.tile([D, RPC], f32)

            # thin operands live entirely in SBUF: [pi, po, f], row = po*128+pi
            u_sb, free_u = tc.tile([128, PO, D], f16, name="u_sb")
            xw_sb, free_xw = tc.tile([128, PO, D], f16, name="xw_sb")
            v_sb, free_v = tc.tile([128, PO, D], f16, name="v_sb")
            w_sb, free_w = tc.tile([128, PO, D], f16, name="w_sb")
            _frees = [free_w, free_v, free_xw, free_u]  # LIFO release order

            def allgather(src, dst, dst_sb):
                nc.gpsimd.collective_compute(
                    "AllGather",
                    mybir.AluOpType.bypass,
                    replica_groups=groups,
                    ins=[src.opt()],
                    outs=[dst.opt()],
                )
                # unpack [c, pi, po_l, f] -> SBUF [pi, c*POL + po_l, f];
                # alternate HWDGE (sync) and SWDGE (gpsimd) so the eight
                # copies drain through two independent DGE paths
                for c in range(N_CORES):
                    eng = nc.sync if c % 2 == 0 else nc.gpsimd
                    eng.dma_start(dst_sb[:, c * POL : (c + 1) * POL, :], dst[c])

            def thin_kxn_producer(sb):
                def produce(nc_, md):
                    return sb[:, md.k_tile_idx * KS : (md.k_tile_idx + 1) * KS, :]

                return produce

            thin_kxm_producer = thin_kxn_producer  # same slicing, TileKxM md

            def sbuf_mxn_producer(sb):
                def produce(nc_, md):
                    t = md.m_tile_idx
                    return sb[:, t * (MB // 128) : (t + 1) * (MB // 128), :]

                return produce

            def relu_evict(nc_, psum, sbuf, md):
                nc_.scalar.activation(
                    sbuf[:], psum[:], mybir.ActivationFunctionType.Relu
                )

            def noop_consumer(nc_, sbuf, md):
                pass

            with (
                tc.tile_pool(name="xT_pool", bufs=3) as pool_xt,
                tc.tile_pool(name="w_pool", bufs=2) as pool_w,
            ):
                # u = relu(x @ W_high), xw = x @ W_conv — evicted straight
                # into SBUF (replicated on every core; no DRAM round trip).
                for sb, wcols, reducer in (
                    (u_sb, slice(0, D), relu_evict),
                    (xw_sb, slice(D, 2 * D), None),
                ):
                    kxm_p, kxm_shape = dma_from_dram_kxm(pool_xt, xT[:])
                    kxn_p, kxn_shape = dma_from_dram_kxn(pool_w, Wcat[:, wcols])
                    composable_matmul_tile_kernel(
                        tc=tc,
                        kxm_shape=kxm_shape,
                        kxn_shape=kxn_shape,
                        output_type=None,
                        kxm_producer=kxm_p,
                        kxn_producer=kxn_p,
                        mxn_consumer=noop_consumer,
                        mxn_subtile_producer=sbuf_mxn_producer(sb),
                        **({"mxn_subtile_reducer": reducer} if reducer else {}),
                        cache_tiles=False,
                    )

            with tc.tile_pool(name="big_stream", bufs=6) as pool_big:

                def big_x_thin(big4, thin_sb, mxn3d):
                    # mxn3d[t]: [pi, po-slice, f] block of the output
                    for t in range(NBLK):
                        kxm_p, kxm_shape = dma_from_dram_kxm(pool_big, big4[t])
                        composable_matmul_tile_kernel(
                            tc=tc,
                            kxm_shape=kxm_shape,
                            kxn_shape=THIN_SHAPE,
                            output_type=f16,
                            kxm_producer=kxm_p,
                            kxn_producer=thin_kxn_producer(thin_sb),
                            mxn_consumer=dma_to_dram_mxn(mxn3d(t)),
                            MAX_K_TILE_SIZE=1024,
                            cache_tiles=False,
                        )

                def thin_x_big(thin_sb, big4, mxn_consumer_fn, blocks=None):
                    # out [D, RPC] fp32, one call per 512-wide column block
                    for t in blocks if blocks is not None else range(NBLK):
                        kxn_p, kxn_shape = dma_from_dram_kxn(pool_big, big4[t])
                        composable_matmul_tile_kernel(
                            tc=tc,
                            kxm_shape=THIN_SHAPE,
                            kxn_shape=kxn_shape,
                            output_type=f32,
                            kxm_producer=thin_kxm_producer(thin_sb),
                            kxn_producer=kxn_p,
                            mxn_consumer=mxn_consumer_fn(t),
                            MAX_K_TILE_SIZE=1024,
                            cache_tiles=False,
                        )

                agg_consumer = lambda t: dma_to_dram_mxn(
                    aggT[:, t * MB : (t + 1) * MB]
                )

                # v_c = d_inv[rows_c] @ u
                big_x_thin(
                    dT4, u_sb,
                    lambda t: v_c4[:, t * (MB // 128) : (t + 1) * (MB // 128), :],
                )

                # aggT = (aL*A[rows_c] @ xw).T = xw.T @ AT — independent of
                # v/w; block 0 is traced here to fill the first AllGather
                # bubble, block 1 after D to fill the second.
                thin_x_big(xw_sb, AT4, agg_consumer, blocks=[0])
                allgather(v_c4, v_sh, v_sb)

                # w_c = (aH * lap)[rows_c] @ v
                big_x_thin(
                    lapT4, v_sb,
                    lambda t: w_c4[:, t * (MB // 128) : (t + 1) * (MB // 128), :],
                )
                thin_x_big(xw_sb, AT4, agg_consumer, blocks=[1])
                allgather(w_c4, w_sh, w_sb)

                # hhT = (d_inv[rows_c] @ w).T = w.T @ dT, fused with the
                # final combine: outT = relu(aggT + b') + hhT.
                with tc.tile_pool(name="combine", bufs=2) as gp:
                    bp_sb, free_bp = tc.tile([128, 2, 1], f32, name="bp_sb")
                    nc.sync.dma_start(
                        bp_sb[:], bp[:].rearrange("(po pi) one -> pi po one", pi=128)
                    )

                    def combine_consumer(t):
                        def consume(nc_, sbuf, md):
                            # sbuf [128, 2, 512] f32 = hhT cols [t*MB,(t+1)*MB)
                            at = gp.tile([128, 2, MB], f32, name="cmb_at")
                            nc_.sync.dma_start(
                                at[:],
                                aggT[:, t * MB : (t + 1) * MB].rearrange(
                                    "(po pi) f -> pi po f", pi=128
                                ),
                            )
                            for s_ in range(2):
                                nc_.scalar.activation(
                                    at[:, s_, :],
                                    at[:, s_, :],
                                    mybir.ActivationFunctionType.Relu,
                                    bias=bp_sb[:, s_, :],
                                )
                            nc_.vector.tensor_add(at[:], at[:], sbuf[:])
                            nc_.sync.dma_start(
                                outT[:, t * MB : (t + 1) * MB].rearrange(
                                    "(po pi) f -> pi po f", pi=128
                                ),
                                at[:],
                            )

                        return consume

                    thin_x_big(w_sb, dT4, combine_consumer)
                    free_bp()

            for f in _frees:
                f()

    _split_excess_waits(nc, mybir)
    return nc


def _get_program():
    if "nc" not in _PROGRAM_CACHE:
        _PROGRAM_CACHE["nc"] = _build_program()
    return _PROGRAM_CACHE["nc"]


def _tile_big(mat_t):
    """[N, RPC] (k-major) -> [NBLK, 128, N//128, MB] fp16 so that each
    [pi, po-slice, :] kxm/kxn tile DMA is contiguous per partition."""
    m16 = np.asarray(mat_t, dtype=np.float16)
    nblk = RPC // MB
    return np.ascontiguousarray(
        m16.reshape(N // 128, 128, nblk, MB).transpose(2, 1, 0, 3)
    )


def _host_prep(x, edge_index, lap, d_inv, W_high, W_conv, b_conv, aL, aH):
    x = np.asarray(x, dtype=np.float32)
    edge_index = np.asarray(edge_index)
    lap = np.asarray(lap, dtype=np.float32)
    d_inv = np.asarray(d_inv, dtype=np.float32)
    W_high = np.asarray(W_high, dtype=np.float32)
    W_conv = np.asarray(W_conv, dtype=np.float32)
    b_conv = np.asarray(b_conv, dtype=np.float32)
    aL = float(np.asarray(aL).reshape(-1)[0])
    aH = float(np.asarray(aH).reshape(-1)[0])

    n = x.shape[0]
    src = edge_index[0].astype(np.int64)
    dst = edge_index[1].astype(np.int64)
    loops = np.arange(n, dtype=np.int64)
    src_all = np.concatenate([src, loops])
    dst_all = np.concatenate([dst, loops])

    deg = np.bincount(dst_all, minlength=n).astype(np.float32)
    dis = np.where(deg > 0, 1.0 / np.sqrt(np.maximum(deg, 1.0)), 0.0).astype(
        np.float32
    )
    # aL folded into the adjacency (aL*relu(y) == relu(aL*y), aL >= 0)
    norm_all = (dis[src_all] * dis[dst_all] * np.float32(aL)).astype(np.float32)

    AT_full = np.zeros((n, n), dtype=np.float32)  # AT[src, dst]
    np.add.at(AT_full, (src_all, dst_all), norm_all)

    bprime = (np.float32(aL) * b_conv).reshape(D, 1).astype(np.float32)
    xT16 = np.ascontiguousarray(x.T.astype(np.float16))  # [256, N]
    Wcat16 = np.ascontiguousarray(
        np.concatenate([W_high, W_conv], axis=1).astype(np.float16)
    )

    in_maps = []
    for c in range(N_CORES):
        rows = slice(c * RPC, (c + 1) * RPC)
        in_maps.append(
            {
                "xT": xT16,
                "Wcat": Wcat16,
                "bp": bprime,
                "dT4": _tile_big(d_inv[rows, :].T),
                "lapT4": _tile_big(lap[rows, :].T * np.float32(aH)),
                "AT4": _tile_big(AT_full[:, rows]),
            }
        )
    return in_maps


def kernel(
    x,
    edge_index,
    lap,
    d_inv,
    W_high,
    W_conv,
    b_conv,
    aL,
    aH,
    _profile=False,
):
    global LAST_EXEC_NS, LAST_RESULTS
    from concourse.bass_utils import run_bass_kernel_spmd

    if _profile:
        _install_ntff_hook()

    in_maps = _host_prep(
        x, edge_index, lap, d_inv, W_high, W_conv, b_conv, aL, aH
    )
    nc = _get_program()
    res = run_bass_kernel_spmd(
        nc, in_maps, list(range(N_CORES)), trace=bool(_profile)
    )
    LAST_EXEC_NS = res.exec_time_ns
    LAST_RESULTS = res
    out = np.concatenate(
        [res.results[c]["outT"].T for c in range(N_CORES)], axis=0
    )
    return np.ascontiguousarray(out.astype(np.float32))
